# revision 5
# baseline (speedup 1.0000x reference)
"""KoLeo-loss Trainium2 kernel, v11 (HW-valid engines only).

walrus rejects TensorScalarPtr on GPSIMD (the simulator allows it), so all
sumsq lands on DVE (STT chunks, t0..3) and Activation (Square with
accumulate, t4..7; the 1283ns activation-table load is prewarmed under the
DMA wait).  Pool and SP only issue DMAs.  Keeps the v9 wins:
  - diag(-xn0) built by one DVE tensor_scalar feeds a K=128 matmul against
    a stride-0 all-ones rhs: the whole transpose/col-row path is gone.
  - per-bank PSUM: broadcasts (start=True zeroes the bank) + BIG*ident on
    block 0 + the col matmul as the closer; two [128,512] abs-min reduces.
  - ident and BIG*ident ship as one host pack; seed-only rsqrt (the value
    jitter is far inside the error budget).
"""

import numpy as np
import ml_dtypes

import concourse.bass as bass
import concourse.mybir as mybir
import concourse.tile as tile
from concourse.bass_utils import run_bass_kernel_spmd

N = 1024
D = 256
NCORES = 8
P = 128
T = N // P
F32 = mybir.dt.float32
BF16 = mybir.dt.bfloat16
I32 = mybir.dt.int32
EPS = 1e-8
BIG = 1e30
MAGIC = 0x5F3759DF

Alu = mybir.AluOpType
Axis = mybir.AxisListType
ActFn = mybir.ActivationFunctionType


def _split_multi_waits(nc: bass.Bass) -> None:
    """Hoist 2+ sync waits per instruction onto carriers (walrus allows 1)."""
    nid = 0
    for bb in nc.m.functions[0].blocks:
        newlist = []
        for inst in bb.instructions:
            si = inst.sync_info
            if si is not None and len(si.on_wait) > 1:
                keep, extra = [si.on_wait[-1]], list(si.on_wait[:-1])
                if inst.engine in (
                    mybir.EngineType.DVE,
                    mybir.EngineType.SP,
                    mybir.EngineType.Activation,
                ):
                    for w in extra:
                        nid += 1
                        carrier = mybir.InstEventSemaphore(
                            name=f"WSPLIT-{nid}", ins=[], outs=[]
                        )
                        carrier.engine = inst.engine
                        carrier.sync_info = mybir.SyncInfo(on_wait=[w], on_update=[])
                        newlist.append(carrier)
                else:
                    for w in extra:
                        placed = False
                        for prev in reversed(newlist):
                            pi = prev.sync_info
                            if pi is not None and any(
                                u.id == w.id for u in pi.on_update
                            ):
                                break
                            if prev.engine != inst.engine:
                                continue
                            if pi is None or len(pi.on_wait) == 0:
                                prev.sync_info = mybir.SyncInfo(
                                    on_wait=[w],
                                    on_update=list(pi.on_update) if pi else [],
                                )
                                placed = True
                                break
                        if not placed:
                            keep.append(w)
                inst.sync_info = mybir.SyncInfo(
                    on_wait=keep, on_update=list(si.on_update)
                )
            newlist.append(inst)
        bb.instructions = newlist


def build_nc() -> bass.Bass:
    nc = bass.Bass()

    xb = nc.dram_tensor("xb", [P, T * D], BF16, kind="ExternalInput")
    idp = nc.dram_tensor("idp", [P, 2 * P], BF16, kind="ExternalInput")
    nn_out = nc.dram_tensor("nn", [P, 2], F32, kind="ExternalOutput")

    with tile.TileContext(nc) as tc:
        with (
            tc.tile_pool(name="sb", bufs=1) as pool,
            tc.tile_pool(name="ps", bufs=1, space="PSUM") as psum,
        ):
            xall = pool.tile([P, T, D], BF16)
            idp_s = pool.tile([P, 2 * P], BF16)
            ones1 = pool.tile([P, 1], BF16)
            s2 = pool.tile([P, T], F32)

            ident_s = idp_s[:, 0:P]
            identB = idp_s[:, P : 2 * P]
            xv = xall[:].rearrange("p t d -> p (t d)")

            def blk(t):
                return xv[:, t * D : (t + 1) * D]

            # --- DMAs: Act s1 [t0,t1] + idp; Pool s2 [t2,t3] + s4 [t6,t7];
            # --- SP s3 [t4,t5].  Consumers: DVE t0..3, Act t4..7.
            nc.scalar.dma_start(xv[:, 0:512], xb[:, 0:512])
            nc.gpsimd.dma_start(xv[:, 512:1024], xb[:, 512:1024])
            nc.gpsimd.dma_start(xv[:, 1536:2048], xb[:, 1536:2048])
            nc.sync.dma_start(xv[:, 1024:1536], xb[:, 1024:1536])
            nc.scalar.dma_start(idp_s[:], idp[:])

            # --- Act: prewarm the activation table under the DMA wait, then
            # --- sumsq t4..7 via Square+accum.
            awarm = pool.tile([1, 1], BF16)
            nc.vector.memset(awarm[:], 1.0)
            nc.vector.memset(ones1[:], 1.0)
            nc.scalar.square(awarm[:], awarm[:])
            ascr = {t: pool.tile([P, D], BF16, name=f"ascr{t}") for t in (5, 6, 7)}
            for t in (6, 7, 5):
                nc.scalar.activation(
                    out=ascr[t][:], in_=blk(t), func=ActFn.Square,
                    accum_out=s2[:, t : t + 1],
                )

            # --- DVE: sumsq t0..3 (STT), A-seed, diagx, B-seed -------------
            dscr = {t: pool.tile([P, D], BF16, name=f"dscr{t}") for t in (0, 1, 2, 3, 4)}

            def dve_sumsq(t):
                nc.vector.scalar_tensor_tensor(
                    out=dscr[t][:], in0=blk(t), scalar=0.0, in1=blk(t),
                    op0=Alu.add, op1=Alu.mult,
                    accum_out=s2[:, t : t + 1],
                )

            for t in (0, 1, 2, 3):
                dve_sumsq(t)
            ish = pool.tile([P, T], I32)
            ybits = pool.tile([P, T], I32)
            xn0 = pool.tile([P, T], BF16)
            loA, loB = slice(0, 4), slice(4, 8)
            nc.vector.tensor_scalar(
                out=ish[:, loA], in0=s2[:, loA].bitcast(I32), scalar1=1,
                scalar2=None, op0=Alu.arith_shift_right,
            )
            nc.vector.tensor_scalar(
                out=ybits[:, loA], in0=ish[:, loA], scalar1=MAGIC, scalar2=-1,
                op0=Alu.subtract, op1=Alu.mult,
            )
            nc.vector.tensor_mul(xn0[:, loA], xall[:, 0:4, 0], ybits[:, loA].bitcast(F32))
            xn0f = pool.tile([P, 1], F32)
            nc.vector.tensor_mul(xn0f[:], xall[:, 0, 0:1], ybits[:, 0:1].bitcast(F32))
            diagx = pool.tile([P, P], BF16)
            nc.vector.tensor_scalar(
                out=diagx[:], in0=ident_s, scalar1=xn0f[:], scalar2=-1.0,
                op0=Alu.mult, op1=Alu.mult,
            )
            dve_sumsq(4)
            nc.vector.tensor_scalar(
                out=ish[:, loB], in0=s2[:, loB].bitcast(I32), scalar1=1,
                scalar2=None, op0=Alu.arith_shift_right,
            )
            nc.vector.tensor_scalar(
                out=ybits[:, loB], in0=ish[:, loB], scalar1=MAGIC, scalar2=-1,
                op0=Alu.subtract, op1=Alu.mult,
            )
            nc.vector.tensor_mul(
                xn0[:, loB], xall[:, 4:8, 0], ybits[:, loB].bitcast(F32)
            )

            # --- PE: bank0 = bcasts, col, diag(stop); bank1 = col, bcasts --
            nn2 = pool.tile([P, 2], F32)
            banks = [psum.tile([P, 512], F32, name=f"B{h}") for h in range(2)]
            pdum = psum.tile([1, 1], BF16)
            nc.tensor.matmul(
                pdum[:], idp_s[0:1, 0:1], idp_s[0:1, 0:1],
                start=True, stop=True, is_transpose=True,
            )
            B0, B1 = banks
            for i in range(4):
                nc.tensor.matmul(
                    B0[:, i * P : (i + 1) * P],
                    xn0[:, i : i + 1].broadcast_to((P, P)),
                    ident_s,
                    start=(i == 0), stop=False,
                )
            nc.tensor.matmul(
                B0[:], diagx[:], ones1[:].broadcast_to((P, 512)),
                start=False, stop=False,
            )
            nc.tensor.matmul(
                B0[:, 0:P], ident_s, identB, start=False, stop=True,
            )
            nc.tensor.matmul(
                B1[:], diagx[:], ones1[:].broadcast_to((P, 512)),
                start=True, stop=False,
            )
            for i in range(4):
                nc.tensor.matmul(
                    B1[:, i * P : (i + 1) * P],
                    xn0[:, 4 + i : 5 + i].broadcast_to((P, P)),
                    ident_s,
                    start=False, stop=(i == 3),
                )
            for h in range(2):
                nc.vector.tensor_reduce(
                    out=nn2[:, h : h + 1], in_=banks[h][:], axis=Axis.X,
                    op=Alu.min, apply_absolute_value=True,
                )
            nc.sync.dma_start(nn_out[:], nn2[:])

    _split_multi_waits(nc)
    return nc


def make_in_maps(X: np.ndarray) -> list[dict[str, np.ndarray]]:
    X = np.asarray(X, dtype=np.float32)
    assert X.shape == (N, D)
    Xb = X.astype(ml_dtypes.bfloat16)
    idp_np = np.zeros((P, 2 * P), dtype=ml_dtypes.bfloat16)
    idp_np[:, 0:P] = np.eye(P)
    idp_np[:, P : 2 * P] = np.eye(P) * ml_dtypes.bfloat16(BIG)
    in_maps = []
    for c in range(NCORES):
        Xr = np.roll(Xb, -P * c, axis=0)
        xb_c = np.ascontiguousarray(
            Xr.reshape(T, P, D).transpose(1, 0, 2).reshape(P, T * D)
        )
        in_maps.append({"xb": xb_c, "idp": idp_np})
    return in_maps


def finish_on_host(nn_blocks: list[np.ndarray]) -> np.ndarray:
    nn = np.concatenate(
        [b.reshape(P, 2).min(axis=1) for b in nn_blocks]
    ).astype(np.float32)
    terms = np.log((nn + np.float32(EPS)).astype(np.float32)).astype(np.float32)
    l0 = np.log(np.float64(np.float32(EPS)))
    total = -(terms.astype(np.float64).sum()) + N * (D - 1) * (-l0)
    return np.asarray(np.float64(total) / N, dtype=np.float32)


_NC_CACHE: bass.Bass | None = None


def _get_nc() -> bass.Bass:
    global _NC_CACHE
    if _NC_CACHE is None:
        _NC_CACHE = build_nc()
    return _NC_CACHE


def kernel(X: np.ndarray) -> np.ndarray:
    nc = _get_nc()
    in_maps = make_in_maps(X)
    res = run_bass_kernel_spmd(nc, in_maps, core_ids=list(range(NCORES))).results
    return finish_on_host([res[c]["nn"] for c in range(NCORES)])
